# revision 8
# baseline (speedup 1.0000x reference)
"""BlockRadiusMixer Trainium2 kernel, batch-major (v2).

Computes, for x [B, 4096] and Q [32, 128, 128]:
    z[b, n, :] = relu(x[b, n*128:(n+1)*128] @ Q[n])
    y = z.reshape(B, 4096);  y /= max(||y||_row, 1e-12)

Data-parallel over 8 NeuronCores (2048 rows each), fp16 I/O (16 MB in +
16 MB out + 1 MB Q per core; the ~320 GB/s practical per-core duplex
HBM rate puts the DMA floor at ~104 us -- fp8 inputs would breach the
2e-2 error gate, est. 3.6e-2, so bytes are fixed).

vs the v1 d-major kernel (133 us): the matmul uses the x block as the
STATIONARY operand (lhsT = x[d, b128], rhs = Q_n[d, e]) so PSUM holds z
as [batch=partitions, features=free] and the whole normalization tail
is partition-local:
  - relu drain PSUM->SBUF fp16 on ScalarE (4 instrs/step of FD=2048;
    ~2.55 us each incl. PSUM access latency -- ScalarE is the busiest
    compute engine at ~12 us/step)
  - sum of relu(z)^2 in ONE DVE scalar_tensor_tensor pass per b-tile:
    out=(z max 0)*z with accum_out = per-partition row sum (fp16 2x
    rate; nc.vector.tensor_tensor_reduce hard-crashes trn2 HW, STT is
    the working equivalent)
  - 1/sqrt(s+eps^2): ACT Sqrt + DVE reciprocal on [128,1], deferred one
    b-tile (TAIL_DEFER) so the Sqrt's wait on the DVE row-sum never
    stalls the ScalarE drain queue
  - y = z * r via DVE tensor_scalar (per-partition scalar, 4 elem/cyc)
  - no ones-matmuls, no cross-partition reduce, no broadcast matmul:
    PE runs only the 64 N=128 block matmuls/step (~6.2 us/step)
Output y is written as [rows, features] so host unshard is a reshape.
Input DMA rides the SP HWDGE ring; output DMA rides the gpsimd SWDGE
ring so write semaphore-waits never block read issue (saves ~7 us).

Measured on 8 axon-tunneled trn2 cores: ~116-118 us/exec (repeat-slope,
all 8 cores), absmax rel err 6.7e-4 vs fp32 reference.  v1 baseline was
~131-133 us.  Component probes: DMA-only ~104.6 us, compute-only
~107 us, PE-only ~50 us, ACT-only ~86 us.
"""

import numpy as np

import concourse.bass as bass
import concourse.tile as tile
from concourse import bacc, mybir
from concourse.bass_utils import run_bass_kernel_spmd

N_CORES = 8
BATCH = 16384
D = 4096
NBLK = 32
BD = 128
B_CORE = BATCH // N_CORES  # 2048
NB = 256  # batch rows per pipeline step (2 b-tiles of 128)
NSTEPS = B_CORE // NB
EPS2 = 1e-24

FP32 = mybir.dt.float32
FP16 = mybir.dt.float16

SPLIT_IN = 2          # input DMAs per step
IN_ALT_RING = False   # odd input splits on ACT HWDGE ring
OUT_SWDGE = True      # output DMA on gpsimd (SWDGE) ring
FB = 16               # feature blocks per PSUM group (16 -> 4 banks)
MM_PSUM_BUFS = 2
DRAIN_DVE = 0         # whole FB-groups per b-tile drained on DVE
DRAIN_K = 0           # blocks per ACT group drained on DVE instead
SQ_ENGINE = "stt"     # "stt" (DVE fused) | "ttr" (broken on HW) | "act"
SQ_CHUNKS = 1         # square+reduce chunks per b-tile
SCALE_CHUNKS = 1      # scale+DMA chunks per b-tile
SCALE_ENG = "vector"  # "vector" | "gpsimd"
OUT_RING = "gpsimd"   # "gpsimd" | "sync" | "scalar"
Q_SPLIT_RING = False  # load Q halves on sync + gpsimd rings
TAIL_DEFER = 1        # b-tiles to delay the sqrt/scale/output tail
SQ_BUFS = 2
XBUFS = 4
ZBUFS = 4
NPOOL_BUFS = 4
LDW_OPT = False       # enable the walrus LDWEIGHTS optimization pass
UNROLL = 1            # rep bodies per hardware-loop iteration
NORM_ENG = "act"      # "dve" (Newton rsqrt, no ACT) | "act" (Sqrt + recip)
BATCH_TAILS = 0       # >0: batch sqrt/recip across this many steps
NEWTON_ITERS = 2


def _apply_ldw_opt_flag():
    from concourse.compiler_utils import get_compiler_flags, set_compiler_flags

    repl = (
        "--internal-backend-options=--enable-neff-debug-info=true "
        "--dump-on-error --enable-ldw-opt=true "
        "--assign-static-dmas-to-sp=false"
    )
    flags = [f for f in get_compiler_flags() if "--internal-backend-options" not in f]
    set_compiler_flags(flags + [repl])


def build_kernel(
    nsteps: int = NSTEPS,
    nb: int = NB,
    repeat: int = 1,
    probe: str | None = None,  # None | "dma" | "compute" | "mm"
):
    if LDW_OPT:
        _apply_ldw_opt_flag()
    assert nb % BD == 0
    ntiles = nb // BD  # b-tiles per step
    nc = bacc.Bacc(
        "TRN2",
        target_bir_lowering=False,
        debug=False,
        enable_asserts=False,
        num_devices=N_CORES,
    )
    xt = nc.dram_tensor(
        "xt", [nsteps, BD, NBLK * nb], FP16, kind="ExternalInput"
    ).ap()
    q = nc.dram_tensor("q", [BD, NBLK * BD], FP16, kind="ExternalInput").ap()
    y = nc.dram_tensor(
        "y", [nsteps, nb, D], FP16, kind="ExternalOutput"
    ).ap()

    ngroups = NBLK // FB  # PSUM groups per b-tile

    with tile.TileContext(nc) as tc:
        with (
            tc.tile_pool(name="qpool", bufs=1) as qpool,
            tc.tile_pool(name="xpool", bufs=XBUFS) as xpool,
            tc.tile_pool(name="zpool", bufs=ZBUFS) as zpool,
            tc.tile_pool(name="sqpool", bufs=SQ_BUFS) as sqpool,
            tc.tile_pool(name="consts", bufs=1) as consts,
            tc.tile_pool(name="npool", bufs=NPOOL_BUFS) as npool,
            tc.tile_pool(name="mm_psum", bufs=MM_PSUM_BUFS, space="PSUM") as mm_psum,
        ):
            eps_c = consts.tile([BD, 1], FP32)
            nc.vector.memset(eps_c[:], EPS2)
            magic_c = consts.tile([BD, 2], mybir.dt.int32)
            nc.vector.memset(magic_c[:], 0x5F3759DF)

            def rep_body():
                # Q in SBUF: partition = d, free = (n, e).  Halves ride
                # different rings so the load overlaps itself.
                q_sb = qpool.tile([BD, NBLK, BD], FP16)
                q_r = q.rearrange("d (n e) -> d n e", e=BD)
                if Q_SPLIT_RING:
                    h = NBLK // 2
                    nc.sync.dma_start(q_sb[:, :h, :], q_r[:, :h, :])
                    nc.gpsimd.dma_start(q_sb[:, h:, :], q_r[:, h:, :])
                else:
                    nc.sync.dma_start(q_sb[:], q_r)

                x_held = None
                pending = []  # (t, j, z_sb, s_ap) per b-tile
                joint_recips = {}  # t -> rec AP (actj mode)

                def emit_tail(tp, j, z_sb, s_ap, r_ap=None):
                    if NORM_ENG == "actj" and r_ap is None:
                        if tp not in joint_recips:
                            s_g = s_ap  # the step's joint [BD, ntiles] s tile
                            nrm_j = npool.tile([BD, ntiles], FP32, tag="nrm_j")
                            nc.scalar.activation(
                                nrm_j[:], s_g[:],
                                mybir.ActivationFunctionType.Sqrt,
                                bias=eps_c[:],
                            )
                            rec_j = npool.tile([BD, ntiles], FP32, tag="rec_j")
                            nc.vector.reciprocal(rec_j[:], nrm_j[:])
                            joint_recips[tp] = rec_j
                        r_ap = joint_recips[tp][:, j : j + 1]
                    if r_ap is None:
                        # r = 1 / sqrt(s + eps^2)
                        nrm = npool.tile([BD, 1], FP32, tag=f"nrm{j}")
                        nc.scalar.activation(
                            nrm[:], s_ap,
                            mybir.ActivationFunctionType.Sqrt,
                            bias=eps_c[:],
                        )
                        recip = npool.tile([BD, 1], FP32, tag=f"r{j}")
                        nc.vector.reciprocal(recip[:], nrm[:])
                        r_ap = recip[:]
                    sc_eng = nc.gpsimd if SCALE_ENG == "gpsimd" else nc.vector
                    o_eng = {
                        "gpsimd": nc.gpsimd,
                        "sync": nc.sync,
                        "scalar": nc.scalar,
                    }[OUT_RING]
                    zf = z_sb[:].rearrange("b n e -> b (n e)")
                    csz = NBLK * BD // SCALE_CHUNKS
                    for cix in range(SCALE_CHUNKS):
                        zc = zf[:, cix * csz : (cix + 1) * csz]
                        sc_eng.tensor_scalar_mul(zc, zc, r_ap)
                        if probe == "compute":
                            if cix == 0:
                                nc.sync.dma_start(
                                    y[tp][j * BD : (j + 1) * BD, :BD],
                                    zc[:, :BD],
                                )
                        else:
                            o_eng.dma_start(
                                y[tp][
                                    j * BD : (j + 1) * BD,
                                    cix * csz : (cix + 1) * csz,
                                ],
                                zc,
                            )

                I32 = mybir.dt.int32

                def emit_rsqrt(s_t, r_t):
                    # r = 1/sqrt(s) via bit-hack seed + Newton iterations
                    t1 = npool.tile([BD, ntiles], I32, tag="rs_t1")
                    nc.vector.tensor_scalar(
                        out=t1[:], in0=s_t[:].bitcast(I32), scalar1=1,
                        scalar2=None, op0=mybir.AluOpType.arith_shift_right,
                    )
                    nc.vector.tensor_tensor(
                        r_t[:].bitcast(I32), magic_c[:, :ntiles], t1[:],
                        mybir.AluOpType.subtract,
                    )
                    r2 = npool.tile([BD, ntiles], FP32, tag="rs_r2")
                    h = npool.tile([BD, ntiles], FP32, tag="rs_h")
                    for _ in range(NEWTON_ITERS):
                        nc.vector.tensor_tensor(
                            r2[:], r_t[:], r_t[:], mybir.AluOpType.mult
                        )
                        nc.vector.tensor_tensor(
                            h[:], s_t[:], r2[:], mybir.AluOpType.mult
                        )
                        nc.vector.tensor_scalar(
                            out=h[:], in0=h[:], scalar1=-0.5, scalar2=1.5,
                            op0=mybir.AluOpType.mult, op1=mybir.AluOpType.add,
                        )
                        nc.vector.tensor_tensor(
                            r_t[:], r_t[:], h[:], mybir.AluOpType.mult
                        )

                grp_steps = []
                grp_pending = []
                s_grp = [None]
                GW = ntiles * max(BATCH_TAILS, 1)

                def flush_group(s_g, steps):
                    nrm_g = npool.tile([BD, GW], FP32, tag="nrm_g")
                    nc.scalar.activation(
                        nrm_g[:], s_g[:],
                        mybir.ActivationFunctionType.Sqrt,
                        bias=eps_c[:],
                    )
                    rec_g = npool.tile([BD, GW], FP32, tag="rec_g")
                    nc.vector.reciprocal(rec_g[:], nrm_g[:])
                    for gi, (tp, zl) in enumerate(steps):
                        for j, z_sb in enumerate(zl):
                            emit_tail(
                                tp, j, z_sb, None,
                                rec_g[:, gi * ntiles + j : gi * ntiles + j + 1],
                            )

                for t in range(nsteps):
                    if probe in ("compute", "mmc", "mmsq", "pe", "act"):
                        if x_held is None:
                            x_held = xpool.tile([BD, NBLK, nb], FP16)
                            nc.sync.dma_start(
                                x_held[:],
                                xt[0].rearrange("d (n b) -> d n b", b=nb),
                            )
                        x_sb = x_held
                    else:
                        x_sb = xpool.tile([BD, NBLK, nb], FP16)
                        xt_t = xt[t].rearrange("d (n b) -> d n b", b=nb)
                        nsp = SPLIT_IN if SPLIT_IN > 1 else 1
                        hn = NBLK // nsp
                        for sp in range(nsp):
                            ieng = (
                                nc.scalar
                                if IN_ALT_RING and sp % 2 == 1
                                else nc.sync
                            )
                            ieng.dma_start(
                                x_sb[:, sp * hn : (sp + 1) * hn, :],
                                xt_t[:, sp * hn : (sp + 1) * hn, :],
                            )
                    if probe == "dma":
                        # stream the input bytes back out with the same
                        # descriptor shape as the real output writes:
                        # [128 partitions, 4096 contiguous] -> 128 rows
                        hb = NBLK // ntiles
                        for c in range(ntiles):
                            eng = nc.gpsimd if OUT_SWDGE else nc.sync
                            eng.dma_start(
                                y[t][c * BD : (c + 1) * BD, :],
                                x_sb[:, c * hb : (c + 1) * hb, :].rearrange(
                                    "d n b -> d (n b)"
                                ),
                            )
                        continue

                    if NORM_ENG in ("dve", "actj") and probe not in ("mm", "mmc", "pe", "act"):
                        s_step = npool.tile([BD, ntiles], FP32, tag="s_step")
                    else:
                        s_step = None
                    z_list = []
                    for j in range(ntiles):
                        # z for rows [t*nb + j*128 ... +128): [b=128, feat 4096]
                        z_sb = zpool.tile([BD, NBLK, BD], FP16, tag=f"z{j}")
                        z_list.append(z_sb)
                        for g in range(ngroups):
                            z_ps = mm_psum.tile([BD, FB, BD], FP32)
                            nmm = 1 if (probe == "act" and not (t == 0 and j == 0)) else FB
                            for h in range(nmm):
                                n = g * FB + h
                                nc.tensor.matmul(
                                    z_ps[:, h, :],
                                    x_sb[:, n, j * BD : (j + 1) * BD],
                                    q_sb[:, n, :],
                                    start=True,
                                    stop=True,
                                )
                            if probe == "pe":
                                # PE-pure: drain a sliver so PSUM recycles
                                nc.scalar.activation(
                                    z_sb[:, g * FB : g * FB + 1, :],
                                    z_ps[:, :1, :],
                                    mybir.ActivationFunctionType.Relu,
                                )
                                continue
                            if probe == "act":
                                # ACT-pure: full drain of a once-filled group
                                nc.scalar.activation(
                                    z_sb[:, g * FB : (g + 1) * FB, :],
                                    z_ps[:],
                                    mybir.ActivationFunctionType.Relu,
                                )
                                continue
                            eng = (
                                nc.vector
                                if g >= ngroups - DRAIN_DVE
                                else nc.scalar
                            )
                            if eng is nc.scalar:
                                sp_k = FB - DRAIN_K
                                nc.scalar.activation(
                                    z_sb[:, g * FB : g * FB + sp_k, :],
                                    z_ps[:, :sp_k, :],
                                    mybir.ActivationFunctionType.Relu,
                                )
                                if DRAIN_K:
                                    nc.vector.tensor_scalar_max(
                                        z_sb[:, g * FB + sp_k : (g + 1) * FB, :],
                                        z_ps[:, sp_k:, :],
                                        0.0,
                                    )
                            else:
                                nc.vector.tensor_scalar_max(
                                    z_sb[:, g * FB : (g + 1) * FB, :],
                                    z_ps[:],
                                    0.0,
                                )
                        if probe in ("mm", "mmc", "pe", "act"):
                            nc.sync.dma_start(
                                y[t][j * BD : (j + 1) * BD, :BD],
                                z_sb[:, 0, :],
                            )
                            continue

                        # s[b] = sum_feat relu(z)^2, fused square+reduce
                        if BATCH_TAILS and probe not in ("mm", "mmc", "pe", "act"):
                            if s_grp[0] is None:
                                s_grp[0] = npool.tile([BD, GW], FP32, tag="s_grp", name="s_grp")
                            gslot = (t % BATCH_TAILS) * ntiles + j
                            s_ap = s_grp[0][:, gslot : gslot + 1]
                        elif s_step is not None:
                            s_ap = s_step[:, j : j + 1]
                        else:
                            s_tile = npool.tile([BD, 1], FP32, tag=f"s{j}")
                            s_ap = s_tile[:]
                        zf = z_sb[:].rearrange("b n e -> b (n e)")
                        if SQ_ENGINE == "pow":
                            # sq = z^2 (z already relu'd), accum = row sum
                            csz = NBLK * BD // SQ_CHUNKS
                            sq = sqpool.tile([BD, csz], FP16, tag=f"sq{j}")
                            parts = []
                            for cix in range(SQ_CHUNKS):
                                zc = zf[:, cix * csz : (cix + 1) * csz]
                                p_ap = (
                                    s_ap
                                    if SQ_CHUNKS == 1
                                    else npool.tile([BD, 1], FP32, tag=f"p{j}{cix}")[:]
                                )
                                nc.vector.tensor_scalar(
                                    out=sq[:],
                                    in0=zc,
                                    scalar1=2.0,
                                    scalar2=None,
                                    op0=mybir.AluOpType.pow,
                                    accum_out=p_ap,
                                )
                                parts.append(p_ap)
                            if SQ_CHUNKS > 1:
                                nc.vector.tensor_tensor(
                                    s_ap, parts[0], parts[1],
                                    mybir.AluOpType.add,
                                )
                        elif SQ_ENGINE == "stt":
                            # (z max 0) * z == relu(z)^2; accum_out = row sum
                            csz = NBLK * BD // SQ_CHUNKS
                            sq = sqpool.tile([BD, csz], FP16, tag=f"sq{j}")
                            parts = []
                            for cix in range(SQ_CHUNKS):
                                zc = zf[:, cix * csz : (cix + 1) * csz]
                                if SQ_CHUNKS == 1:
                                    p_ap = s_ap
                                else:
                                    p_tile = npool.tile(
                                        [BD, 1], FP32, tag=f"p{j}{cix}"
                                    )
                                    p_ap = p_tile[:]
                                nc.vector.scalar_tensor_tensor(
                                    out=sq[:],
                                    in0=zc,
                                    scalar=0.0,
                                    in1=zc,
                                    op0=mybir.AluOpType.max,
                                    op1=mybir.AluOpType.mult,
                                    accum_out=p_ap,
                                )
                                parts.append(p_ap)
                            if SQ_CHUNKS > 1:
                                nc.vector.tensor_tensor(
                                    s_ap, parts[0], parts[1],
                                    mybir.AluOpType.add,
                                )
                        elif SQ_ENGINE == "ttr":
                            csz = NBLK * BD // SQ_CHUNKS
                            sq = sqpool.tile([BD, csz], FP16, tag=f"sq{j}")
                            for cix in range(SQ_CHUNKS):
                                zc = zf[:, cix * csz : (cix + 1) * csz]
                                nc.vector.tensor_tensor_reduce(
                                    out=sq[:],
                                    in0=zc,
                                    in1=zc,
                                    scale=1.0,
                                    scalar=0.0 if cix == 0 else s_ap,
                                    op0=mybir.AluOpType.mult,
                                    op1=mybir.AluOpType.add,
                                    accum_out=s_ap,
                                )
                        else:
                            csz = NBLK * BD // SQ_CHUNKS
                            sq = sqpool.tile([BD, csz], FP16, tag=f"sq{j}")
                            parts = []
                            for cix in range(SQ_CHUNKS):
                                zc = zf[:, cix * csz : (cix + 1) * csz]
                                p_ap = npool.tile([BD, 1], FP32, tag=f"p{j}{cix}")
                                nc.scalar.activation(
                                    sq[:],
                                    zc,
                                    mybir.ActivationFunctionType.Square,
                                    accum_out=p_ap[:],
                                )
                                parts.append(p_ap)
                            nc.vector.tensor_tensor(
                                s_ap, parts[0][:], parts[1][:],
                                mybir.AluOpType.add,
                            )

                        if probe == "mmsq":
                            continue
                        if BATCH_TAILS:
                            pass  # handled at group level below
                        elif NORM_ENG == "actj":
                            pending.append((t, j, z_sb, s_step, None))
                            if len(pending) > TAIL_DEFER:
                                emit_tail(*pending.pop(0))
                        elif NORM_ENG != "dve":
                            pending.append((t, j, z_sb, s_ap, None))
                            if len(pending) > TAIL_DEFER:
                                emit_tail(*pending.pop(0))

                    if BATCH_TAILS and probe not in ("mm", "mmc", "mmsq", "pe", "act"):
                        grp_steps.append((t, z_list))
                        if (t + 1) % BATCH_TAILS == 0:
                            grp_pending.append((s_grp[0], grp_steps))
                            grp_steps = []
                            s_grp[0] = None
                            if len(grp_pending) > 1:
                                flush_group(*grp_pending.pop(0))

                    if NORM_ENG == "dve" and probe not in ("mm", "mmc", "mmsq", "pe", "act"):
                        r_step = npool.tile([BD, ntiles], FP32, tag="r_step")
                        emit_rsqrt(s_step, r_step)
                        for j in range(ntiles):
                            pending.append(
                                (t, j, z_list[j], None, r_step[:, j : j + 1])
                            )
                            if len(pending) > TAIL_DEFER:
                                emit_tail(*pending.pop(0))

                for args in pending:
                    emit_tail(*args)
                pending.clear()
                for gargs in grp_pending:
                    flush_group(*gargs)
                grp_pending.clear()

            hint = () if probe == "dma" else (mybir.EngineType.PE,)
            if repeat == 1:
                rep_body()
            else:
                unroll = UNROLL if repeat % UNROLL == 0 else 1
                with tc.For_i(0, repeat // unroll, 1, hint_engines=hint):
                    for _ in range(unroll):
                        rep_body()

    nc.compile()
    return nc


_NC_CACHE: dict = {}


def _get_nc():
    if "nc" not in _NC_CACHE:
        _NC_CACHE["nc"] = build_kernel()
    return _NC_CACHE["nc"]


def shard_inputs(x: np.ndarray, Q: np.ndarray) -> list[dict]:
    """Per-core input maps, pre-tiled so every DMA is contiguous:
    xtt[t, d, n*nb + b] = x_shard[t*NB + b, n*128 + d]."""
    host_dt = np.dtype(np.float16)
    x = np.asarray(x, dtype=np.float32)
    Q = np.asarray(Q, dtype=np.float32)
    qh = np.ascontiguousarray(
        Q.transpose(1, 0, 2).astype(host_dt)
    ).reshape(BD, NBLK * BD)
    xs = x.reshape(N_CORES, NSTEPS, NB, NBLK, BD)  # [c, t, b, n, d]
    in_maps = []
    for c in range(N_CORES):
        xtt = np.ascontiguousarray(
            xs[c].transpose(0, 3, 2, 1).astype(host_dt)
        )  # [t, d, n, b]
        in_maps.append({"xt": xtt.reshape(NSTEPS, BD, NBLK * NB), "q": qh})
    return in_maps


def unshard_output(results: list[dict]) -> np.ndarray:
    out = np.empty((N_CORES, B_CORE, D), dtype=np.float32)
    for c in range(N_CORES):
        out[c] = results[c]["y"].reshape(B_CORE, D).astype(np.float32)
    return out.reshape(BATCH, D)


def kernel(x, Q) -> np.ndarray:
    nc = _get_nc()
    in_maps = shard_inputs(x, Q)
    res = run_bass_kernel_spmd(nc, in_maps, core_ids=list(range(N_CORES)))
    return unshard_output(res.results)
